# revision 27
# baseline (speedup 1.0000x reference)
"""Trainium2 Bass kernel: 3x3 stride-1 pad-1 Conv2D, NCHW, via 1D Winograd.

v6: device-side input transform, split image-0 startup. See kernel.py
docstring for the algorithm description.
"""

import numpy as np

import concourse.bass as bass
import concourse.mybir as mybir
import concourse.tile as tile
from concourse import bacc
from concourse.bass_utils import run_bass_kernel_spmd

N_CORES = 8
N_FULL = 32
N_PER_CORE = N_FULL // N_CORES  # 4
CIN = 128
COUT = 256
H = W = 56
HP = WP = 58  # padded spatial
NPAIR = H // 2  # 28 row-pairs per image
QB = 7  # row-pairs per block
NB = NPAIR // QB  # 4 blocks per image
NF = QB * W  # 392 matmul free dim (pairs x width)
F32 = mybir.dt.float32
F16 = mybir.dt.float16

TRACE = False
LAST_RESULT = None

_prog = None


def _build_program():
    nc = bacc.Bacc("TRN2", target_bir_lowering=False, debug=False)
    x_d = nc.declare_dram_parameter("x", [N_PER_CORE, CIN, HP * WP], F16, isOutput=False)
    w_d = nc.declare_dram_parameter("wt", [CIN, 24 * 128], F16, isOutput=False)
    out_d = nc.declare_dram_parameter(
        "out", [N_PER_CORE, 2, 128, H * W], F16, isOutput=True
    )

    AluOp = mybir.AluOpType
    ActFn = mybir.ActivationFunctionType

    POS_ORDER = (1, 2, 0, 3)
    T_DEFS = (
        (0, 2, AluOp.subtract),  # t0 = d0 - d2
        (1, 2, AluOp.add),       # t1 = d1 + d2
        (2, 1, AluOp.subtract),  # t2 = d2 - d1
        (1, 3, AluOp.subtract),  # t3 = d1 - d3
    )

    with tile.TileContext(nc) as tc:
        with (
            tc.tile_pool(name="const", bufs=1) as const_pool,
            tc.tile_pool(name="xin", bufs=2) as x_pool,
            tc.tile_pool(name="tin", bufs=2) as t_pool,
            tc.tile_pool(name="mc", bufs=3) as mc_pool,
            tc.tile_pool(name="sd", bufs=3) as sd_pool,
            tc.tile_pool(name="outp", bufs=4) as out_pool,
            tc.tile_pool(name="psum", bufs=8, space="PSUM") as psum_pool,
        ):
            w_sbs = {}
            for c in range(2):
                for pos in range(4):
                    w_cp = const_pool.tile([CIN, 3 * 128], F16, tag=f"w{c}p{pos}")
                    w_sbs[(c, pos)] = w_cp

            def load_w(c, pos):
                base = (c * 4 + pos) * 3 * 128
                halfw = 3 * 128 // 2
                for eng, lo, hi in ((nc.sync, 0, halfw), (nc.scalar, halfw, 3 * 128)):
                    eng.dma_start(
                        out=w_sbs[(c, pos)][:, lo:hi], in_=w_d[:, base + lo : base + hi]
                    )

            x_view = x_d[:].rearrange("n p (h w) -> n p h w", w=WP)
            x_tiles = {}

            def load_image(i, engs=None, r0=0, r1=HP):
                if i in x_tiles:
                    x_c = x_tiles[i]
                else:
                    x_c = x_pool.tile([CIN, HP, WP], F16, name=f"x{i}")
                    x_tiles[i] = x_c
                e0, e1 = engs or (nc.sync, nc.scalar)
                mid = (r0 + r1) // 2
                for eng, lo, hi in ((e0, r0, mid), (e1, mid, r1)):
                    eng.dma_start(out=x_c[:, lo:hi, :], in_=x_view[i][:, lo:hi, :])

            scratch = const_pool.tile([128, NF], F16)
            nc.vector.memset(scratch[:], 0.0)
            warm_ps = psum_pool.tile([128, NF], F32, tag="ps")
            NWARM = 12
            for wi in range(NWARM):
                nc.tensor.matmul(
                    warm_ps[:], lhsT=scratch[:, :128], rhs=scratch[:],
                    start=(wi == 0), stop=(wi == NWARM - 1), skip_group_check=True,
                )

            # x0's lower half (rows for pairs 0..13) split across BOTH
            # queues lands ~10.5us; the c0 weight chunks follow immediately
            # instead of queueing behind a whole-image transfer.
            load_image(0, r0=0, r1=30)
            load_w(0, 1)
            load_w(0, 2)
            load_w(0, 0)
            load_w(0, 3)
            load_image(0, r0=30, r1=HP)
            load_w(1, 1)
            load_w(1, 2)
            load_w(1, 0)
            load_w(1, 3)
            # Images 1 and 2 ride the otherwise-idle gpsimd/vector DMA
            # queues so they land early without delaying weights/stores.
            load_image(1, engs=(nc.gpsimd, nc.gpsimd))
            load_image(2, engs=(nc.gpsimd, nc.gpsimd))

            t_tiles = {}

            def transform_image(i, p0=0, p1=NPAIR):
                xt = x_tiles[i]
                if i in t_tiles:
                    t_t = t_tiles[i]
                else:
                    t_t = t_pool.tile([CIN, 4, NPAIR, WP], F16, name=f"t{i}")
                    t_tiles[i] = t_t
                E = 2 * (p1 - p0) - 1
                for pos in POS_ORDER:
                    a0, a1, op = T_DEFS[pos]
                    nc.vector.tensor_tensor(
                        t_t[:, pos, p0:p1],
                        xt[:, 2 * p0 + a0 : 2 * p0 + a0 + E : 2, :],
                        xt[:, 2 * p0 + a1 : 2 * p0 + a1 + E : 2, :],
                        op,
                    )

            transform_image(0, 0, 14)
            transform_image(0, 14, NPAIR)

            store_ctr = [0]

            def compute_block(i, b, q0=0, q1=QB):
                t_t = t_tiles[i]
                nq = q1 - q0
                nf = nq * W
                for c in range(2):
                    ms = {}
                    for pos in POS_ORDER:
                        ps = psum_pool.tile([128, NF], F32, tag="ps")
                        ps_v = ps[:, :nf].rearrange("p (q w) -> p q w", w=W)
                        for kw in range(3):
                            nc.tensor.matmul(
                                ps_v,
                                lhsT=w_sbs[(c, pos)][:, kw * 128 : (kw + 1) * 128],
                                rhs=t_t[:, pos, b * QB + q0 : b * QB + q1, kw : kw + W],
                                start=(kw == 0), stop=(kw == 2),
                            )
                        ms[pos] = ps
                    out_t = out_pool.tile([128, QB, 2, W], F16)
                    sl = slice(0, nf)
                    mc = mc_pool.tile([128, 2, NF], F16)
                    nc.scalar.activation(mc[:, 0, sl], ms[1][:, sl], ActFn.Copy)
                    nc.scalar.activation(mc[:, 1, sl], ms[2][:, sl], ActFn.Copy)
                    s_t = sd_pool.tile([128, NF], F16, tag="s")
                    d_t = sd_pool.tile([128, NF], F16, tag="d")
                    nc.gpsimd.tensor_tensor(
                        s_t[:, sl], mc[:, 0, sl], mc[:, 1, sl], AluOp.add
                    )
                    nc.vector.tensor_tensor(
                        d_t[:, sl], mc[:, 0, sl], mc[:, 1, sl], AluOp.subtract
                    )
                    for j, m_ps, sd_t, op0, scl in (
                        (0, ms[0], s_t, AluOp.bypass, 0.0),
                        (1, ms[3], d_t, AluOp.mult, -1.0),
                    ):
                        nc.vector.scalar_tensor_tensor(
                            out_t[:, :nq, j, :],
                            m_ps[:, :nf].rearrange("p (q w) -> p q w", w=W),
                            scl,
                            sd_t[:, :nf].rearrange("p (q w) -> p q w", w=W),
                            op0,
                            AluOp.add,
                        )
                    lo = (b * QB + q0) * 2 * W
                    store_eng = nc.sync if store_ctr[0] % 2 == 0 else nc.scalar
                    store_ctr[0] += 1
                    store_eng.dma_start(
                        out=out_d[i, c][:, lo : lo + nq * 2 * W],
                        in_=out_t[:, :nq].rearrange("p q j w -> p (q j w)"),
                    )

            for i in range(N_PER_CORE):
                if i + 2 < N_PER_CORE and (i + 2) not in x_tiles:
                    load_image(i + 2)
                for b in range(NB):
                    if b == 1 and i + 1 < N_PER_CORE:
                        transform_image(i + 1)
                    if i == N_PER_CORE - 1 and b == NB - 1:
                        compute_block(i, b, 0, 4)
                        compute_block(i, b, 4, QB)
                    else:
                        compute_block(i, b)
                del x_tiles[i], t_tiles[i]
    nc.compile()
    return nc


_G = np.array(
    [[1.0, 0.0, 0.0], [0.5, 0.5, 0.5], [0.5, -0.5, 0.5], [0.0, 0.0, 1.0]],
    dtype=np.float64,
)


def kernel(x: np.ndarray, weight: np.ndarray, bias: np.ndarray) -> np.ndarray:
    global _prog, LAST_RESULT
    x = np.ascontiguousarray(x, dtype=np.float32)
    weight = np.ascontiguousarray(weight, dtype=np.float32)
    bias = np.ascontiguousarray(bias, dtype=np.float32)

    x_pad = np.zeros((N_FULL, CIN, HP, WP), dtype=np.float16)
    x_pad[:, :, 1:-1, 1:-1] = x
    x_pad = x_pad.reshape(N_FULL, CIN, HP * WP)

    u = np.einsum("ph,oihw->oipw", _G, weight.astype(np.float64))
    wt = np.ascontiguousarray(
        u.reshape(2, 128, CIN, 4, 3).transpose(2, 0, 3, 4, 1).reshape(CIN, 24 * 128)
    ).astype(np.float16)

    if _prog is None:
        _prog = _build_program()

    in_maps = [
        {
            "x": np.ascontiguousarray(x_pad[i * N_PER_CORE : (i + 1) * N_PER_CORE]),
            "wt": wt,
        }
        for i in range(N_CORES)
    ]
    res = run_bass_kernel_spmd(_prog, in_maps, list(range(N_CORES)), trace=TRACE)
    LAST_RESULT = res
    out = np.concatenate([r["out"] for r in res.results], axis=0)
    out = out.astype(np.float32).reshape(N_FULL, COUT, H, W)
    if bias.any():
        out += bias[None, :, None, None]
    return out


# revision 28
# speedup vs baseline: 1.0109x; 1.0109x over previous
"""Trainium2 Bass kernel: 3x3 stride-1 pad-1 Conv2D, NCHW, via 1D Winograd.

v6: device-side input transform, split image-0 startup. See kernel.py
docstring for the algorithm description.
"""

import numpy as np

import concourse.bass as bass
import concourse.mybir as mybir
import concourse.tile as tile
from concourse import bacc
from concourse.bass_utils import run_bass_kernel_spmd

N_CORES = 8
N_FULL = 32
N_PER_CORE = N_FULL // N_CORES  # 4
CIN = 128
COUT = 256
H = W = 56
HP = WP = 58  # padded spatial
NPAIR = H // 2  # 28 row-pairs per image
QB = 7  # row-pairs per block
NB = NPAIR // QB  # 4 blocks per image
NF = QB * W  # 392 matmul free dim (pairs x width)
F32 = mybir.dt.float32
F16 = mybir.dt.float16

TRACE = False
LAST_RESULT = None

_prog = None


def _build_program():
    nc = bacc.Bacc("TRN2", target_bir_lowering=False, debug=False)
    x_d = nc.declare_dram_parameter("x", [N_PER_CORE, CIN, HP * WP], F16, isOutput=False)
    w_d = nc.declare_dram_parameter("wt", [CIN, 24 * 128], F16, isOutput=False)
    out_d = nc.declare_dram_parameter(
        "out", [N_PER_CORE, 2, 128, H * W], F16, isOutput=True
    )

    AluOp = mybir.AluOpType
    ActFn = mybir.ActivationFunctionType

    POS_ORDER = (1, 2, 0, 3)
    T_DEFS = (
        (0, 2, AluOp.subtract),  # t0 = d0 - d2
        (1, 2, AluOp.add),       # t1 = d1 + d2
        (2, 1, AluOp.subtract),  # t2 = d2 - d1
        (1, 3, AluOp.subtract),  # t3 = d1 - d3
    )

    with tile.TileContext(nc) as tc:
        with (
            tc.tile_pool(name="const", bufs=1) as const_pool,
            tc.tile_pool(name="xin", bufs=2) as x_pool,
            tc.tile_pool(name="tin", bufs=2) as t_pool,
            tc.tile_pool(name="mc", bufs=3) as mc_pool,
            tc.tile_pool(name="sd", bufs=3) as sd_pool,
            tc.tile_pool(name="outp", bufs=4) as out_pool,
            tc.tile_pool(name="psum", bufs=8, space="PSUM") as psum_pool,
        ):
            w_sbs = {}
            for c in range(2):
                for pos in range(4):
                    w_cp = const_pool.tile([CIN, 3 * 128], F16, tag=f"w{c}p{pos}")
                    w_sbs[(c, pos)] = w_cp

            def load_w(c, pos):
                base = (c * 4 + pos) * 3 * 128
                halfw = 3 * 128 // 2
                for eng, lo, hi in ((nc.sync, 0, halfw), (nc.scalar, halfw, 3 * 128)):
                    eng.dma_start(
                        out=w_sbs[(c, pos)][:, lo:hi], in_=w_d[:, base + lo : base + hi]
                    )

            x_view = x_d[:].rearrange("n p (h w) -> n p h w", w=WP)
            x_tiles = {}

            def load_image(i, engs=None, r0=0, r1=HP):
                if i in x_tiles:
                    x_c = x_tiles[i]
                else:
                    x_c = x_pool.tile([CIN, HP, WP], F16, name=f"x{i}")
                    x_tiles[i] = x_c
                e0, e1 = engs or (nc.sync, nc.scalar)
                mid = (r0 + r1) // 2
                for eng, lo, hi in ((e0, r0, mid), (e1, mid, r1)):
                    eng.dma_start(out=x_c[:, lo:hi, :], in_=x_view[i][:, lo:hi, :])

            scratch = const_pool.tile([128, NF], F16)
            nc.vector.memset(scratch[:], 0.0)
            warm_ps = psum_pool.tile([128, NF], F32, tag="ps")
            NWARM = 12
            for wi in range(NWARM):
                nc.tensor.matmul(
                    warm_ps[:], lhsT=scratch[:, :128], rhs=scratch[:],
                    start=(wi == 0), stop=(wi == NWARM - 1), skip_group_check=True,
                )

            # x0's lower half (rows for pairs 0..13) split across BOTH
            # queues lands ~10.5us; the c0 weight chunks follow immediately
            # instead of queueing behind a whole-image transfer.
            load_image(0, r0=0, r1=30)
            load_w(0, 1)
            load_w(0, 2)
            load_w(0, 0)
            load_w(0, 3)
            load_image(0, r0=30, r1=HP)
            load_w(1, 1)
            load_w(1, 2)
            load_w(1, 0)
            load_w(1, 3)
            # Images 1 and 2 ride the otherwise-idle gpsimd/vector DMA
            # queues so they land early without delaying weights/stores.
            load_image(1, engs=(nc.gpsimd, nc.gpsimd))
            load_image(2, engs=(nc.gpsimd, nc.gpsimd))

            t_tiles = {}

            def transform_image(i, p0=0, p1=NPAIR):
                xt = x_tiles[i]
                if i in t_tiles:
                    t_t = t_tiles[i]
                else:
                    t_t = t_pool.tile([CIN, 4, NPAIR, WP], F16, name=f"t{i}")
                    t_tiles[i] = t_t
                E = 2 * (p1 - p0) - 1
                for pos in POS_ORDER:
                    a0, a1, op = T_DEFS[pos]
                    nc.vector.tensor_tensor(
                        t_t[:, pos, p0:p1],
                        xt[:, 2 * p0 + a0 : 2 * p0 + a0 + E : 2, :],
                        xt[:, 2 * p0 + a1 : 2 * p0 + a1 + E : 2, :],
                        op,
                    )

            transform_image(0, 0, 14)

            store_ctr = [0]

            def compute_block(i, b, q0=0, q1=QB, fast_tail=False):
                t_t = t_tiles[i]
                nq = q1 - q0
                nf = nq * W
                for c in range(2):
                    ms = {}
                    for pos in POS_ORDER:
                        ps = psum_pool.tile([128, NF], F32, tag="ps")
                        ps_v = ps[:, :nf].rearrange("p (q w) -> p q w", w=W)
                        for kw in range(3):
                            nc.tensor.matmul(
                                ps_v,
                                lhsT=w_sbs[(c, pos)][:, kw * 128 : (kw + 1) * 128],
                                rhs=t_t[:, pos, b * QB + q0 : b * QB + q1, kw : kw + W],
                                start=(kw == 0), stop=(kw == 2),
                            )
                        ms[pos] = ps
                    out_t = out_pool.tile([128, QB, 2, W], F16)
                    sl = slice(0, nf)
                    mc = mc_pool.tile([128, 2, NF], F16)
                    nc.scalar.activation(mc[:, 0, sl], ms[1][:, sl], ActFn.Copy)
                    nc.scalar.activation(mc[:, 1, sl], ms[2][:, sl], ActFn.Copy)
                    s_t = sd_pool.tile([128, NF], F16, tag="s")
                    d_t = sd_pool.tile([128, NF], F16, tag="d")
                    s_eng = nc.vector if fast_tail else nc.gpsimd
                    s_eng.tensor_tensor(
                        s_t[:, sl], mc[:, 0, sl], mc[:, 1, sl], AluOp.add
                    )
                    nc.vector.tensor_tensor(
                        d_t[:, sl], mc[:, 0, sl], mc[:, 1, sl], AluOp.subtract
                    )
                    for j, m_ps, sd_t, op0, scl in (
                        (0, ms[0], s_t, AluOp.bypass, 0.0),
                        (1, ms[3], d_t, AluOp.mult, -1.0),
                    ):
                        nc.vector.scalar_tensor_tensor(
                            out_t[:, :nq, j, :],
                            m_ps[:, :nf].rearrange("p (q w) -> p q w", w=W),
                            scl,
                            sd_t[:, :nf].rearrange("p (q w) -> p q w", w=W),
                            op0,
                            AluOp.add,
                        )
                    lo = (b * QB + q0) * 2 * W
                    if fast_tail:
                        h = nq // 2 + 1
                        for eng, a0, a1 in ((nc.sync, 0, h), (nc.scalar, h, nq)):
                            eng.dma_start(
                                out=out_d[i, c][:, lo + a0 * 2 * W : lo + a1 * 2 * W],
                                in_=out_t[:, a0:a1].rearrange("p q j w -> p (q j w)"),
                            )
                    else:
                        store_eng = nc.sync if store_ctr[0] % 2 == 0 else nc.scalar
                        store_ctr[0] += 1
                        store_eng.dma_start(
                            out=out_d[i, c][:, lo : lo + nq * 2 * W],
                            in_=out_t[:, :nq].rearrange("p q j w -> p (q j w)"),
                        )

            for i in range(N_PER_CORE):
                if i + 2 < N_PER_CORE and (i + 2) not in x_tiles:
                    load_image(i + 2)
                for b in range(NB):
                    if b == 1 and i + 1 < N_PER_CORE:
                        transform_image(i + 1)
                    if i == N_PER_CORE - 1 and b == NB - 1:
                        compute_block(i, b, 0, 4)
                        compute_block(i, b, 4, QB, fast_tail=True)
                    else:
                        compute_block(i, b)
                    if i == 0 and b == 0:
                        transform_image(0, 14, NPAIR)
                del x_tiles[i], t_tiles[i]
    nc.compile()
    return nc


_G = np.array(
    [[1.0, 0.0, 0.0], [0.5, 0.5, 0.5], [0.5, -0.5, 0.5], [0.0, 0.0, 1.0]],
    dtype=np.float64,
)


def kernel(x: np.ndarray, weight: np.ndarray, bias: np.ndarray) -> np.ndarray:
    global _prog, LAST_RESULT
    x = np.ascontiguousarray(x, dtype=np.float32)
    weight = np.ascontiguousarray(weight, dtype=np.float32)
    bias = np.ascontiguousarray(bias, dtype=np.float32)

    x_pad = np.zeros((N_FULL, CIN, HP, WP), dtype=np.float16)
    x_pad[:, :, 1:-1, 1:-1] = x
    x_pad = x_pad.reshape(N_FULL, CIN, HP * WP)

    u = np.einsum("ph,oihw->oipw", _G, weight.astype(np.float64))
    wt = np.ascontiguousarray(
        u.reshape(2, 128, CIN, 4, 3).transpose(2, 0, 3, 4, 1).reshape(CIN, 24 * 128)
    ).astype(np.float16)

    if _prog is None:
        _prog = _build_program()

    in_maps = [
        {
            "x": np.ascontiguousarray(x_pad[i * N_PER_CORE : (i + 1) * N_PER_CORE]),
            "wt": wt,
        }
        for i in range(N_CORES)
    ]
    res = run_bass_kernel_spmd(_prog, in_maps, list(range(N_CORES)), trace=TRACE)
    LAST_RESULT = res
    out = np.concatenate([r["out"] for r in res.results], axis=0)
    out = out.astype(np.float32).reshape(N_FULL, COUT, H, W)
    if bias.any():
        out += bias[None, :, None, None]
    return out


# revision 29
# speedup vs baseline: 1.0362x; 1.0251x over previous
"""Trainium2 Bass kernel: 3x3 stride-1 pad-1 Conv2D, NCHW, via 1D Winograd.

v6: device-side input transform, split image-0 startup. See kernel.py
docstring for the algorithm description.
"""

import numpy as np

import concourse.bass as bass
import concourse.mybir as mybir
import concourse.tile as tile
from concourse import bacc
from concourse.bass_utils import run_bass_kernel_spmd

N_CORES = 8
N_FULL = 32
N_PER_CORE = N_FULL // N_CORES  # 4
CIN = 128
COUT = 256
H = W = 56
HP = WP = 58  # padded spatial
NPAIR = H // 2  # 28 row-pairs per image
QB = 7  # row-pairs per block
NB = NPAIR // QB  # 4 blocks per image
NF = QB * W  # 392 matmul free dim (pairs x width)
F32 = mybir.dt.float32
F16 = mybir.dt.float16

TRACE = False
LAST_RESULT = None

_prog = None


def _build_program():
    nc = bacc.Bacc("TRN2", target_bir_lowering=False, debug=False)
    x_d = nc.declare_dram_parameter("x", [N_PER_CORE, CIN, HP * WP], F16, isOutput=False)
    w_d = nc.declare_dram_parameter("wt", [CIN, 24 * 128], F16, isOutput=False)
    out_d = nc.declare_dram_parameter(
        "out", [N_PER_CORE, 2, 128, H * W], F16, isOutput=True
    )

    AluOp = mybir.AluOpType
    ActFn = mybir.ActivationFunctionType

    POS_ORDER = (1, 2, 0, 3)
    T_DEFS = (
        (0, 2, AluOp.subtract),  # t0 = d0 - d2
        (1, 2, AluOp.add),       # t1 = d1 + d2
        (2, 1, AluOp.subtract),  # t2 = d2 - d1
        (1, 3, AluOp.subtract),  # t3 = d1 - d3
    )

    with tile.TileContext(nc) as tc:
        with (
            tc.tile_pool(name="const", bufs=1) as const_pool,
            tc.tile_pool(name="xin", bufs=2) as x_pool,
            tc.tile_pool(name="tin", bufs=2) as t_pool,
            tc.tile_pool(name="mc", bufs=3) as mc_pool,
            tc.tile_pool(name="sd", bufs=3) as sd_pool,
            tc.tile_pool(name="outp", bufs=4) as out_pool,
            tc.tile_pool(name="psum", bufs=8, space="PSUM") as psum_pool,
        ):
            w_sbs = {}
            for c in range(2):
                for pos in range(4):
                    w_cp = const_pool.tile([CIN, 3 * 128], F16, tag=f"w{c}p{pos}")
                    w_sbs[(c, pos)] = w_cp

            def load_w(c, pos):
                base = (c * 4 + pos) * 3 * 128
                halfw = 3 * 128 // 2
                for eng, lo, hi in ((nc.sync, 0, halfw), (nc.scalar, halfw, 3 * 128)):
                    eng.dma_start(
                        out=w_sbs[(c, pos)][:, lo:hi], in_=w_d[:, base + lo : base + hi]
                    )

            x_view = x_d[:].rearrange("n p (h w) -> n p h w", w=WP)
            x_tiles = {}

            def load_image(i, engs=None, r0=0, r1=HP):
                if i in x_tiles:
                    x_c = x_tiles[i]
                else:
                    x_c = x_pool.tile([CIN, HP, WP], F16, name=f"x{i}")
                    x_tiles[i] = x_c
                e0, e1 = engs or (nc.sync, nc.scalar)
                mid = (r0 + r1) // 2
                for eng, lo, hi in ((e0, r0, mid), (e1, mid, r1)):
                    eng.dma_start(out=x_c[:, lo:hi, :], in_=x_view[i][:, lo:hi, :])

            scratch = const_pool.tile([128, NF], F16)
            nc.vector.memset(scratch[:], 0.0)
            warm_ps = psum_pool.tile([128, NF], F32, tag="ps")
            NWARM = 12
            for wi in range(NWARM):
                nc.tensor.matmul(
                    warm_ps[:], lhsT=scratch[:, :128], rhs=scratch[:],
                    start=(wi == 0), stop=(wi == NWARM - 1), skip_group_check=True,
                )

            # x0's lower half (rows for pairs 0..13) split across BOTH
            # queues lands ~10.5us; the c0 weight chunks follow immediately
            # instead of queueing behind a whole-image transfer.
            load_image(0, r0=0, r1=30)
            load_w(0, 1)
            load_w(0, 2)
            load_w(0, 0)
            load_w(0, 3)
            load_image(0, r0=30, r1=HP)
            load_w(1, 1)
            load_w(1, 2)
            load_w(1, 0)
            load_w(1, 3)
            # Images 1 and 2 ride the otherwise-idle gpsimd/vector DMA
            # queues so they land early without delaying weights/stores.
            load_image(1, engs=(nc.gpsimd, nc.gpsimd))
            load_image(2, engs=(nc.gpsimd, nc.gpsimd))

            t_tiles = {}

            def transform_image(i, p0=0, p1=NPAIR):
                xt = x_tiles[i]
                if i in t_tiles:
                    t_t = t_tiles[i]
                else:
                    t_t = t_pool.tile([CIN, 4, NPAIR, WP], F16, name=f"t{i}")
                    t_tiles[i] = t_t
                E = 2 * (p1 - p0) - 1
                for pos in POS_ORDER:
                    a0, a1, op = T_DEFS[pos]
                    nc.vector.tensor_tensor(
                        t_t[:, pos, p0:p1],
                        xt[:, 2 * p0 + a0 : 2 * p0 + a0 + E : 2, :],
                        xt[:, 2 * p0 + a1 : 2 * p0 + a1 + E : 2, :],
                        op,
                    )

            transform_image(0, 0, 14)
            transform_image(0, 14, NPAIR)

            store_ctr = [0]

            def compute_block(i, b, q0=0, q1=QB):
                t_t = t_tiles[i]
                nq = q1 - q0
                nf = nq * W
                for c in range(2):
                    ms = {}
                    for pos in POS_ORDER:
                        ps = psum_pool.tile([128, NF], F32, tag="ps")
                        ps_v = ps[:, :nf].rearrange("p (q w) -> p q w", w=W)
                        for kw in range(3):
                            nc.tensor.matmul(
                                ps_v,
                                lhsT=w_sbs[(c, pos)][:, kw * 128 : (kw + 1) * 128],
                                rhs=t_t[:, pos, b * QB + q0 : b * QB + q1, kw : kw + W],
                                start=(kw == 0), stop=(kw == 2),
                            )
                        ms[pos] = ps
                    out_t = out_pool.tile([128, QB, 2, W], F16)
                    sl = slice(0, nf)
                    mc = mc_pool.tile([128, 2, NF], F16)
                    nc.scalar.activation(mc[:, 0, sl], ms[1][:, sl], ActFn.Copy)
                    nc.scalar.activation(mc[:, 1, sl], ms[2][:, sl], ActFn.Copy)
                    s_t = sd_pool.tile([128, NF], F16, tag="s")
                    d_t = sd_pool.tile([128, NF], F16, tag="d")
                    nc.gpsimd.tensor_tensor(
                        s_t[:, sl], mc[:, 0, sl], mc[:, 1, sl], AluOp.add
                    )
                    nc.vector.tensor_tensor(
                        d_t[:, sl], mc[:, 0, sl], mc[:, 1, sl], AluOp.subtract
                    )
                    for j, m_ps, sd_t, op0, scl in (
                        (0, ms[0], s_t, AluOp.bypass, 0.0),
                        (1, ms[3], d_t, AluOp.mult, -1.0),
                    ):
                        nc.vector.scalar_tensor_tensor(
                            out_t[:, :nq, j, :],
                            m_ps[:, :nf].rearrange("p (q w) -> p q w", w=W),
                            scl,
                            sd_t[:, :nf].rearrange("p (q w) -> p q w", w=W),
                            op0,
                            AluOp.add,
                        )
                    lo = (b * QB + q0) * 2 * W
                    store_eng = nc.sync if store_ctr[0] % 2 == 0 else nc.scalar
                    store_ctr[0] += 1
                    store_eng.dma_start(
                        out=out_d[i, c][:, lo : lo + nq * 2 * W],
                        in_=out_t[:, :nq].rearrange("p q j w -> p (q j w)"),
                    )

            for i in range(N_PER_CORE):
                if i + 2 < N_PER_CORE and (i + 2) not in x_tiles:
                    load_image(i + 2)
                for b in range(NB):
                    if b == 1 and i + 1 < N_PER_CORE:
                        transform_image(i + 1)
                    if i == N_PER_CORE - 1 and b == NB - 1:
                        compute_block(i, b, 0, 4)
                        compute_block(i, b, 4, QB)
                    else:
                        compute_block(i, b)
                del x_tiles[i], t_tiles[i]
    nc.compile()
    return nc


_G = np.array(
    [[1.0, 0.0, 0.0], [0.5, 0.5, 0.5], [0.5, -0.5, 0.5], [0.0, 0.0, 1.0]],
    dtype=np.float64,
)


def kernel(x: np.ndarray, weight: np.ndarray, bias: np.ndarray) -> np.ndarray:
    global _prog, LAST_RESULT
    x = np.ascontiguousarray(x, dtype=np.float32)
    weight = np.ascontiguousarray(weight, dtype=np.float32)
    bias = np.ascontiguousarray(bias, dtype=np.float32)

    x_pad = np.zeros((N_FULL, CIN, HP, WP), dtype=np.float16)
    x_pad[:, :, 1:-1, 1:-1] = x
    x_pad = x_pad.reshape(N_FULL, CIN, HP * WP)

    u = np.einsum("ph,oihw->oipw", _G, weight.astype(np.float64))
    wt = np.ascontiguousarray(
        u.reshape(2, 128, CIN, 4, 3).transpose(2, 0, 3, 4, 1).reshape(CIN, 24 * 128)
    ).astype(np.float16)

    if _prog is None:
        _prog = _build_program()

    in_maps = [
        {
            "x": np.ascontiguousarray(x_pad[i * N_PER_CORE : (i + 1) * N_PER_CORE]),
            "wt": wt,
        }
        for i in range(N_CORES)
    ]
    res = run_bass_kernel_spmd(_prog, in_maps, list(range(N_CORES)), trace=TRACE)
    LAST_RESULT = res
    out = np.concatenate([r["out"] for r in res.results], axis=0)
    out = out.astype(np.float32).reshape(N_FULL, COUT, H, W)
    if bias.any():
        out += bias[None, :, None, None]
    return out
